# revision 30
# baseline (speedup 1.0000x reference)
"""Masked attention-weight kernel (dense_transformer) for 8 TRN2 NeuronCores.

Computes, for inputs query/key [32,1024,512] f32, masks [32,1024] i32:
    q = relu(query @ Wq + bq); k = relu(key @ Wk + bk)
    w = softmax((q @ k^T)/sqrt(512) + key_mask_additive) * query_mask
Output: [32, 1024, 1024] f32.

Strategy: pure data-parallel over batch (4 batches/core, no collectives).
Host pre-transposes query/key to [B_local, D, L] and casts to bf16 so every
device matmul is transpose-free; compute is bf16 with f32 PSUM accumulation.
Softmax skips max-subtraction (logits bounded ~+-12; exp is safe in f32).
The key mask is applied additively (-1e6) to the post-relu k-projection,
which makes masked logits ~-2e8 so exp underflows to exactly 0, and the
ACT exp's fused accum_out produces the masked row sum for free.

Per-core pipeline, per batch:
  1. kT[e,j] = relu(Wk.T @ keyT + bk) via PE matmuls -> ACT relu+bias,
     then +mask on DVE/GpSimd (batch 0 runs the matmuls dt-major so the PE
     consumes (wk_dt, xk_dt) DMA pairs in arrival order at cold start).
  2. qT[e,i] likewise.
  3. For each 128-row block: S = qT.T @ kTm (PE, f32 psum), ACT exp with
     fused row-sum, DVE reciprocal * query_mask, DVE scale, DMA out
     (stores alternate between the gpsimd and scalar queues).
"""

import sys

sys.path.insert(0, "/opt/trn_rl_repo")

import numpy as np
import ml_dtypes
from contextlib import ExitStack

import concourse.tile as tile
from concourse import bacc, mybir
from concourse.bass_utils import run_bass_kernel_spmd

P = 128
B, LQ, LK, D = 32, 1024, 1024, 512
NCORES = 8
BL = B // NCORES          # batches per core
NDT = D // P              # contraction tiles for projections
NET = D // P              # output-feature tiles (= S contraction tiles)
NIB = LQ // P             # 128-row blocks of S per batch
NH = LK // 512            # 512-col halves
SCALE = float(1.0 / np.sqrt(D))
MASKC = -1.0e6

F32 = mybir.dt.float32
BF16 = mybir.dt.bfloat16
AF = mybir.ActivationFunctionType

_CACHE = {}


def _body(tc, qT, kT, Wq, Wk, bq, bk, maskb, qm, out):
    nc = tc.nc
    with ExitStack() as ctx:
        consts = ctx.enter_context(tc.tile_pool(name="consts", bufs=1))
        wpool = ctx.enter_context(tc.tile_pool(name="w", bufs=1))
        inpool = ctx.enter_context(tc.tile_pool(name="inp", bufs=2))
        actpool = ctx.enter_context(tc.tile_pool(name="act", bufs=2))
        mpool = ctx.enter_context(tc.tile_pool(name="mask", bufs=2))
        epool = ctx.enter_context(tc.tile_pool(name="exp", bufs=3))
        opool = ctx.enter_context(tc.tile_pool(name="pout", bufs=3))
        stpool = ctx.enter_context(tc.tile_pool(name="stat", bufs=6))
        ppsum = ctx.enter_context(tc.tile_pool(name="ppsum", bufs=2, space="PSUM"))
        spsum = ctx.enter_context(tc.tile_pool(name="spsum", bufs=3, space="PSUM"))

        # Weights on the scalar DMA queue, inputs on sync, small tensors on
        # gpsimd — three queues pull concurrently at cold start.
        wk_sb = [wpool.tile([P, D], BF16, tag=f"wk{dt_}", name=f"wk{dt_}")
                 for dt_ in range(NDT)]
        wq_sb = [wpool.tile([P, D], BF16, tag=f"wq{dt_}", name=f"wq{dt_}")
                 for dt_ in range(NDT)]
        for dt_ in range(NDT):
            nc.scalar.dma_start(
                out=wk_sb[dt_][:], in_=Wk[dt_ * P:(dt_ + 1) * P, :])
        for dt_ in range(NDT):
            nc.scalar.dma_start(
                out=wq_sb[dt_][:], in_=Wq[dt_ * P:(dt_ + 1) * P, :])

        bk_sb = consts.tile([P, NET], F32)
        nc.gpsimd.dma_start(out=bk_sb[:], in_=bk[:])
        bq_sb = consts.tile([P, NET], F32)
        nc.gpsimd.dma_start(out=bq_sb[:], in_=bq[:])

        # PE warmup: 8 dummy matmuls (~3.4us of cold PE busy, exactly one
        # HAM activity window) on scratch tiles while the first input DMAs
        # are in flight, so the clock-gate reaches K=8/8 just before real
        # matmuls start. Results are never read.
        warm_in = consts.tile([P, 512], BF16, name="warm_in")
        nc.vector.memset(warm_in[:], 0.0)
        warm_ps = ppsum.tile([P, 512], F32, tag="proj", name="warm_ps")
        for _ in range(8):
            nc.tensor.matmul(
                warm_ps[:], lhsT=warm_in[:, 0:P], rhs=warm_in[:],
                start=True, stop=True,
            )

        def load_inputs(b):
            xk, xq = [], []
            for dt_ in range(NDT):
                t = inpool.tile([P, LK], BF16, tag=f"xk{dt_}")
                if b == 0 and dt_ == 0:
                    # split so the very first matmul's 128KB dep lands sooner
                    for h in range(NH):
                        nc.sync.dma_start(
                            out=t[:, h * 512:(h + 1) * 512],
                            in_=kT[b, 0:P, h * 512:(h + 1) * 512])
                else:
                    nc.sync.dma_start(
                        out=t[:], in_=kT[b, dt_ * P:(dt_ + 1) * P, :])
                xk.append(t)
            mask_sb = mpool.tile([P, LK], BF16, tag="maskb")
            if b > 0:
                # prefetched with plenty of slack; keep off the sync queue
                nc.gpsimd.dma_start(out=mask_sb[:], in_=maskb[b])
            for dt_ in range(NDT):
                t = inpool.tile([P, LQ], BF16, tag=f"xq{dt_}")
                if b == 0 and dt_ == 0:
                    for h in range(NH):
                        nc.sync.dma_start(
                            out=t[:, h * 512:(h + 1) * 512],
                            in_=qT[b, 0:P, h * 512:(h + 1) * 512])
                else:
                    nc.sync.dma_start(
                        out=t[:], in_=qT[b, dt_ * P:(dt_ + 1) * P, :])
                xq.append(t)
            if b == 0:
                # batch 0: issue after xq so the mask transfer doesn't steal
                # bandwidth from the cold-start critical path (wk/xk pairs)
                nc.sync.dma_start(out=mask_sb[:], in_=maskb[b])
            qm_sb = mpool.tile([P, NIB], F32, tag="qm")
            nc.gpsimd.dma_start(out=qm_sb[:], in_=qm[b])
            return xk, mask_sb, xq, qm_sb

        def relu_epilogue(ps, bias_sb, out_tiles, et, ih, on_dve=False):
            if on_dve:
                # (psum + bias) max 0 — exact relu+bias as one DVE op
                nc.vector.tensor_scalar(
                    out=out_tiles[et][:, ih * 512:(ih + 1) * 512],
                    in0=ps[:],
                    scalar1=bias_sb[:, et:et + 1],
                    scalar2=0.0,
                    op0=mybir.AluOpType.add,
                    op1=mybir.AluOpType.max,
                )
            else:
                nc.scalar.activation(
                    out=out_tiles[et][:, ih * 512:(ih + 1) * 512],
                    in_=ps[:],
                    func=AF.Relu,
                    bias=bias_sb[:, et:et + 1],
                    scale=1.0,
                )

        def proj(xin, w_sb, bias_sb, out_tiles):
            # out_tiles[et] = relu(W[:, et].T @ x + b)
            for et in range(NET):
                for ih in range(NH):
                    ps = ppsum.tile([P, 512], F32, tag="proj")
                    for dt_ in range(NDT):
                        nc.tensor.matmul(
                            ps[:],
                            lhsT=w_sb[dt_][:, et * P:(et + 1) * P],
                            rhs=xin[dt_][:, ih * 512:(ih + 1) * 512],
                            start=(dt_ == 0),
                            stop=(dt_ == NDT - 1),
                        )
                    relu_epilogue(ps, bias_sb, out_tiles, et, ih)

        def proj_coldstart(xin, w_sb, bias_sb, out_tiles, pfx="coldk", epi_ih_major=False, split_epi=False):
            # Batch-0 k-proj only: dt-major order so the PE consumes
            # (wk_dt, xk_dt) DMA pairs in arrival order instead of stalling
            # on wk1-3; all 4 et accumulation groups are open at once,
            # borrowing the (still idle) S-phase psum pool for et 0-2.
            pss = []
            for et in range(NET - 1):
                t = spsum.tile([P, LK], F32, tag="S", name=f"{pfx}ps{et}")
                pss.append([t[:, 0:512], t[:, 512:1024]])
            pss.append([ppsum.tile([P, 512], F32, tag="proj", name=f"{pfx}3a")[:],
                        ppsum.tile([P, 512], F32, tag="proj", name=f"{pfx}3b")[:]])
            for dt_ in range(NDT):
                for et in range(NET):
                    for ih in range(NH):
                        nc.tensor.matmul(
                            pss[et][ih],
                            lhsT=w_sb[dt_][:, et * P:(et + 1) * P],
                            rhs=xin[dt_][:, ih * 512:(ih + 1) * 512],
                            start=(dt_ == 0),
                            stop=(dt_ == NDT - 1),
                        )
            # epi_ih_major: S block 0 needs only the ih=0 halves of qT,
            # so drain those four groups first
            if epi_ih_major:
                order = [(et, ih) for ih in range(NH) for et in range(NET)]
            else:
                order = [(et, ih) for et in range(NET) for ih in range(NH)]
            for n, (et, ih) in enumerate(order):
                relu_epilogue(pss[et][ih], bias_sb, out_tiles, et, ih,
                              on_dve=(split_epi and n % 2 == 1))

        def mask_add(kraw, mask_sb, b):
            kTm = [actpool.tile([P, LK], BF16, tag=f"kTm{et}",
                                name=f"kTm{et}_{b}")
                   for et in range(NET)]
            for et in range(NET):
                # split across gpsimd and vector so neither gates the S phase
                eng = nc.gpsimd if et % 2 == 0 else nc.vector
                eng.tensor_add(kTm[et][:], kraw[et][:], mask_sb[:])
            return kTm

        def s_block(b, ib, qTt, kTm, qm_sb):
            sp = spsum.tile([P, LK], F32, tag="S")
            for et in range(NET):
                for jh in range(NH):
                    nc.tensor.matmul(
                        sp[:, jh * 512:(jh + 1) * 512],
                        lhsT=qTt[et][:, ib * P:(ib + 1) * P],
                        rhs=kTm[et][:, jh * 512:(jh + 1) * 512],
                        start=(et == 0),
                        stop=(et == NET - 1),
                    )
            ex = epool.tile([P, LK], BF16, tag="exp")
            rs = stpool.tile([P, 1], F32, tag="rowsum")
            nc.scalar.activation(
                out=ex[:], in_=sp[:], func=AF.Exp, scale=SCALE,
                accum_out=rs[:],
            )
            rc = stpool.tile([P, 1], F32, tag="recip")
            nc.vector.reciprocal(out=rc[:], in_=rs[:])
            rq = stpool.tile([P, 1], F32, tag="rq")
            nc.vector.tensor_tensor(
                out=rq[:], in0=rc[:], in1=qm_sb[:, ib:ib + 1],
                op=mybir.AluOpType.mult,
            )
            po = opool.tile([P, LK], F32, tag="po")
            nc.vector.tensor_scalar(
                out=po[:], in0=ex[:], scalar1=rq[:], scalar2=None,
                op0=mybir.AluOpType.mult,
            )
            # alternate store queues so the output backlog drains 2x faster
            # (sync, not scalar: scalar's ACT must not stall behind DMA issue)
            eng = nc.gpsimd if ib % 2 == 0 else nc.sync
            eng.dma_start(out=out[b, ib * P:(ib + 1) * P, :], in_=po[:])

        def s_block_final(b, ib, qTt, kTm, qm_sb):
            # Last block of the kernel: jh-major matmuls into two separate
            # 1-bank psums + a fully split epilogue (independent half tiles)
            # so the first half's exp/mul/store overlap the second half's
            # matmuls and exp — shortening the serial tail after the last MM.
            sps = [ppsum.tile([P, 512], F32, tag="proj", name=f"fsp{jh}")
                   for jh in range(NH)]
            rss = [stpool.tile([P, 1], F32, tag=f"rowsum{jh}", name=f"frs{jh}")
                   for jh in range(NH)]
            exs = [epool.tile([P, 512], BF16, tag=f"fex{jh}", name=f"fex{jh}")
                   for jh in range(NH)]
            for jh in range(NH):
                for et in range(NET):
                    nc.tensor.matmul(
                        sps[jh][:],
                        lhsT=qTt[et][:, ib * P:(ib + 1) * P],
                        rhs=kTm[et][:, jh * 512:(jh + 1) * 512],
                        start=(et == 0),
                        stop=(et == NET - 1),
                    )
                nc.scalar.activation(
                    out=exs[jh][:], in_=sps[jh][:],
                    func=AF.Exp, scale=SCALE, accum_out=rss[jh][:],
                )
            rs = stpool.tile([P, 1], F32, tag="rowsumt")
            nc.vector.tensor_tensor(
                out=rs[:], in0=rss[0][:], in1=rss[1][:],
                op=mybir.AluOpType.add)
            rc = stpool.tile([P, 1], F32, tag="recip")
            nc.vector.reciprocal(out=rc[:], in_=rs[:])
            rq = stpool.tile([P, 1], F32, tag="rq")
            nc.vector.tensor_tensor(
                out=rq[:], in0=rc[:], in1=qm_sb[:, ib:ib + 1],
                op=mybir.AluOpType.mult,
            )
            for jh in range(NH):
                poh = opool.tile([P, 512], F32, tag=f"fpo{jh}", name=f"fpo{jh}")
                nc.vector.tensor_scalar(
                    out=poh[:], in0=exs[jh][:],
                    scalar1=rq[:], scalar2=None,
                    op0=mybir.AluOpType.mult,
                )
                eng = nc.gpsimd if jh == 0 else nc.sync
                eng.dma_start(
                    out=out[b, ib * P:(ib + 1) * P, jh * 512:(jh + 1) * 512],
                    in_=poh[:],
                )

        def s_phase(b, qTt, kTm, qm_sb):
            for ib in range(NIB):
                if b == BL - 1 and ib == NIB - 1:
                    s_block_final(b, ib, qTt, kTm, qm_sb)
                else:
                    s_block(b, ib, qTt, kTm, qm_sb)

        cur = load_inputs(0)
        for b in range(BL):
            xk, mask_sb, xq, qm_sb = cur
            kraw = [actpool.tile([P, LK], BF16, tag=f"kraw{et}",
                                 name=f"kraw{et}_{b}")
                    for et in range(NET)]
            if b == 0:
                proj_coldstart(xk, wk_sb, bk_sb, kraw, pfx="coldk")
            else:
                proj(xk, wk_sb, bk_sb, kraw)
            kTm = mask_add(kraw, mask_sb, b)
            qTt = [actpool.tile([P, LQ], BF16, tag=f"qT{et}",
                                name=f"qT{et}_{b}")
                   for et in range(NET)]
            if b == 0:
                proj_coldstart(xq, wq_sb, bq_sb, qTt, pfx="coldq", split_epi=True)
            else:
                proj(xq, wq_sb, bq_sb, qTt)
            if b + 1 < BL:
                cur = load_inputs(b + 1)
            s_phase(b, qTt, kTm, qm_sb)


def _build():
    nc = bacc.Bacc(
        "TRN2",
        target_bir_lowering=False,
        debug=False,
        enable_asserts=False,
        num_devices=NCORES,
    )
    qT = nc.dram_tensor("qT", [BL, D, LQ], BF16, kind="ExternalInput").ap()
    kT = nc.dram_tensor("kT", [BL, D, LK], BF16, kind="ExternalInput").ap()
    Wq = nc.dram_tensor("Wq", [D, D], BF16, kind="ExternalInput").ap()
    Wk = nc.dram_tensor("Wk", [D, D], BF16, kind="ExternalInput").ap()
    bq = nc.dram_tensor("bq", [P, NET], F32, kind="ExternalInput").ap()
    bk = nc.dram_tensor("bk", [P, NET], F32, kind="ExternalInput").ap()
    maskb = nc.dram_tensor("maskb", [BL, P, LK], BF16, kind="ExternalInput").ap()
    qm = nc.dram_tensor("qm", [BL, P, NIB], F32, kind="ExternalInput").ap()
    out = nc.dram_tensor("out", [BL, LQ, LK], F32, kind="ExternalOutput").ap()

    with tile.TileContext(nc) as tc:
        _body(tc, qT, kT, Wq, Wk, bq, bk, maskb, qm, out)
    nc.compile()
    return nc


def _get_nc():
    if "nc" not in _CACHE:
        _CACHE["nc"] = _build()
    return _CACHE["nc"]


def _make_in_maps(query, key, query_mask, key_mask, Wq, bq, Wk, bk):
    bf = ml_dtypes.bfloat16
    query = np.asarray(query, dtype=np.float32)
    key = np.asarray(key, dtype=np.float32)
    query_mask = np.asarray(query_mask)
    key_mask = np.asarray(key_mask)
    Wq_b = np.asarray(Wq, dtype=np.float32).astype(bf)
    Wk_b = np.asarray(Wk, dtype=np.float32).astype(bf)
    # bias for feature e lives at partition e%128, column e//128
    bq_t = np.asarray(bq, dtype=np.float32).reshape(NET, P).T.copy()
    bk_t = np.asarray(bk, dtype=np.float32).reshape(NET, P).T.copy()

    in_maps = []
    for c in range(NCORES):
        sl = slice(c * BL, (c + 1) * BL)
        qTc = query[sl].transpose(0, 2, 1).astype(bf)
        kTc = key[sl].transpose(0, 2, 1).astype(bf)
        mrow = (MASKC * (1 - key_mask[sl])).astype(bf)            # [BL, LK]
        maskb = np.ascontiguousarray(
            np.broadcast_to(mrow[:, None, :], (BL, P, LK))
        )
        qmc = (
            query_mask[sl].astype(np.float32)
            .reshape(BL, NIB, P).transpose(0, 2, 1).copy()
        )
        in_maps.append({
            "qT": qTc, "kT": kTc, "Wq": Wq_b, "Wk": Wk_b,
            "bq": bq_t, "bk": bk_t, "maskb": maskb, "qm": qmc,
        })
    return in_maps


def run(query, key, query_mask, key_mask, Wq, bq, Wk, bk, **kwargs):
    """Run on hardware; returns (output, BassKernelResults)."""
    nc = _get_nc()
    in_maps = _make_in_maps(query, key, query_mask, key_mask, Wq, bq, Wk, bk)
    res = run_bass_kernel_spmd(nc, in_maps, core_ids=list(range(NCORES)), **kwargs)
    outs = [res.results[c]["out"] for c in range(NCORES)]
    full = np.concatenate(outs, axis=0).astype(np.float32, copy=False)
    return full, res


def kernel(query, key, query_mask, key_mask, Wq, bq, Wk, bk):
    full, _ = run(query, key, query_mask, key_mask, Wq, bq, Wk, bk)
    return full


# revision 31
# speedup vs baseline: 1.1171x; 1.1171x over previous
"""Masked attention-weight kernel (dense_transformer) for 8 TRN2 NeuronCores.

Computes, for inputs query/key [32,1024,512] f32, masks [32,1024] i32:
    q = relu(query @ Wq + bq); k = relu(key @ Wk + bk)
    w = softmax((q @ k^T)/sqrt(512) + key_mask_additive) * query_mask
Output: [32, 1024, 1024] f32.

Strategy: pure data-parallel over batch (4 batches/core, no collectives).
Host pre-transposes query/key to [B_local, D, L] and casts to bf16 so every
device matmul is transpose-free; compute is bf16 with f32 PSUM accumulation.
Softmax skips max-subtraction (logits bounded ~+-12; exp is safe in f32).
The key mask is applied additively (-1e6) to the post-relu k-projection,
which makes masked logits ~-2e8 so exp underflows to exactly 0, and the
ACT exp's fused accum_out produces the masked row sum for free.

Per-core pipeline, per batch:
  1. kT[e,j] = relu(Wk.T @ keyT + bk) via PE matmuls -> ACT relu+bias,
     then +mask on DVE/GpSimd (batch 0 runs the matmuls dt-major so the PE
     consumes (wk_dt, xk_dt) DMA pairs in arrival order at cold start).
  2. qT[e,i] likewise.
  3. For each 128-row block: S = qT.T @ kTm (PE, f32 psum), ACT exp with
     fused row-sum, DVE reciprocal * query_mask, DVE scale, DMA out
     (stores alternate between the gpsimd and scalar queues).
"""

import sys

sys.path.insert(0, "/opt/trn_rl_repo")

import numpy as np
import ml_dtypes
from contextlib import ExitStack

import concourse.tile as tile
from concourse import bacc, mybir
from concourse.bass_utils import run_bass_kernel_spmd

P = 128
B, LQ, LK, D = 32, 1024, 1024, 512
NCORES = 8
BL = B // NCORES          # batches per core
NDT = D // P              # contraction tiles for projections
NET = D // P              # output-feature tiles (= S contraction tiles)
NIB = LQ // P             # 128-row blocks of S per batch
NH = LK // 512            # 512-col halves
SCALE = float(1.0 / np.sqrt(D))
MASKC = -1.0e6

F32 = mybir.dt.float32
BF16 = mybir.dt.bfloat16
AF = mybir.ActivationFunctionType

_CACHE = {}


def _body(tc, qT, kT, Wq, Wk, bq, bk, maskb, qm, out):
    nc = tc.nc
    with ExitStack() as ctx:
        consts = ctx.enter_context(tc.tile_pool(name="consts", bufs=1))
        wpool = ctx.enter_context(tc.tile_pool(name="w", bufs=1))
        inpool = ctx.enter_context(tc.tile_pool(name="inp", bufs=3))
        actpool = ctx.enter_context(tc.tile_pool(name="act", bufs=2))
        mpool = ctx.enter_context(tc.tile_pool(name="mask", bufs=2))
        epool = ctx.enter_context(tc.tile_pool(name="exp", bufs=3))
        opool = ctx.enter_context(tc.tile_pool(name="pout", bufs=3))
        stpool = ctx.enter_context(tc.tile_pool(name="stat", bufs=6))
        ppsum = ctx.enter_context(tc.tile_pool(name="ppsum", bufs=2, space="PSUM"))
        spsum = ctx.enter_context(tc.tile_pool(name="spsum", bufs=3, space="PSUM"))

        # Weights on the scalar DMA queue, inputs on sync, small tensors on
        # gpsimd — three queues pull concurrently at cold start.
        wk_sb = [wpool.tile([P, D], BF16, tag=f"wk{dt_}", name=f"wk{dt_}")
                 for dt_ in range(NDT)]
        wq_sb = [wpool.tile([P, D], BF16, tag=f"wq{dt_}", name=f"wq{dt_}")
                 for dt_ in range(NDT)]
        for dt_ in range(NDT):
            nc.scalar.dma_start(
                out=wk_sb[dt_][:], in_=Wk[dt_ * P:(dt_ + 1) * P, :])
        for dt_ in range(NDT):
            nc.scalar.dma_start(
                out=wq_sb[dt_][:], in_=Wq[dt_ * P:(dt_ + 1) * P, :])

        bk_sb = consts.tile([P, NET], F32)
        nc.gpsimd.dma_start(out=bk_sb[:], in_=bk[:])
        bq_sb = consts.tile([P, NET], F32)
        nc.gpsimd.dma_start(out=bq_sb[:], in_=bq[:])

        # PE warmup: 8 dummy matmuls (~3.4us of cold PE busy, exactly one
        # HAM activity window) on scratch tiles while the first input DMAs
        # are in flight, so the clock-gate reaches K=8/8 just before real
        # matmuls start. Results are never read.
        warm_in = consts.tile([P, 512], BF16, name="warm_in")
        nc.vector.memset(warm_in[:], 0.0)
        warm_ps = ppsum.tile([P, 512], F32, tag="proj", name="warm_ps")
        for _ in range(8):
            nc.tensor.matmul(
                warm_ps[:], lhsT=warm_in[:, 0:P], rhs=warm_in[:],
                start=True, stop=True,
            )

        def load_inputs(b):
            xk, xq = [], []
            for dt_ in range(NDT):
                t = inpool.tile([P, LK], BF16, tag=f"xk{dt_}")
                if b == 0 and dt_ == 0:
                    # split so the very first matmul's 128KB dep lands sooner
                    for h in range(NH):
                        nc.sync.dma_start(
                            out=t[:, h * 512:(h + 1) * 512],
                            in_=kT[b, 0:P, h * 512:(h + 1) * 512])
                else:
                    nc.sync.dma_start(
                        out=t[:], in_=kT[b, dt_ * P:(dt_ + 1) * P, :])
                xk.append(t)
            mask_sb = mpool.tile([P, LK], BF16, tag="maskb")
            if b > 0:
                # prefetched with plenty of slack; keep off the sync queue
                nc.gpsimd.dma_start(out=mask_sb[:], in_=maskb[b])
            for dt_ in range(NDT):
                t = inpool.tile([P, LQ], BF16, tag=f"xq{dt_}")
                if b == 0 and dt_ == 0:
                    for h in range(NH):
                        nc.sync.dma_start(
                            out=t[:, h * 512:(h + 1) * 512],
                            in_=qT[b, 0:P, h * 512:(h + 1) * 512])
                else:
                    nc.sync.dma_start(
                        out=t[:], in_=qT[b, dt_ * P:(dt_ + 1) * P, :])
                xq.append(t)
            if b == 0:
                # batch 0: issue after xq so the mask transfer doesn't steal
                # bandwidth from the cold-start critical path (wk/xk pairs)
                nc.sync.dma_start(out=mask_sb[:], in_=maskb[b])
            qm_sb = mpool.tile([P, NIB], F32, tag="qm")
            nc.gpsimd.dma_start(out=qm_sb[:], in_=qm[b])
            return xk, mask_sb, xq, qm_sb

        def relu_epilogue(ps, bias_sb, out_tiles, et, ih, on_dve=False):
            if on_dve:
                # (psum + bias) max 0 — exact relu+bias as one DVE op
                nc.vector.tensor_scalar(
                    out=out_tiles[et][:, ih * 512:(ih + 1) * 512],
                    in0=ps[:],
                    scalar1=bias_sb[:, et:et + 1],
                    scalar2=0.0,
                    op0=mybir.AluOpType.add,
                    op1=mybir.AluOpType.max,
                )
            else:
                nc.scalar.activation(
                    out=out_tiles[et][:, ih * 512:(ih + 1) * 512],
                    in_=ps[:],
                    func=AF.Relu,
                    bias=bias_sb[:, et:et + 1],
                    scale=1.0,
                )

        def proj(xin, w_sb, bias_sb, out_tiles):
            # out_tiles[et] = relu(W[:, et].T @ x + b)
            for et in range(NET):
                for ih in range(NH):
                    ps = ppsum.tile([P, 512], F32, tag="proj")
                    for dt_ in range(NDT):
                        nc.tensor.matmul(
                            ps[:],
                            lhsT=w_sb[dt_][:, et * P:(et + 1) * P],
                            rhs=xin[dt_][:, ih * 512:(ih + 1) * 512],
                            start=(dt_ == 0),
                            stop=(dt_ == NDT - 1),
                        )
                    relu_epilogue(ps, bias_sb, out_tiles, et, ih)

        def proj_coldstart(xin, w_sb, bias_sb, out_tiles, pfx="coldk", epi_ih_major=False, split_epi=False):
            # Batch-0 k-proj only: dt-major order so the PE consumes
            # (wk_dt, xk_dt) DMA pairs in arrival order instead of stalling
            # on wk1-3; all 4 et accumulation groups are open at once,
            # borrowing the (still idle) S-phase psum pool for et 0-2.
            pss = []
            for et in range(NET - 1):
                t = spsum.tile([P, LK], F32, tag="S", name=f"{pfx}ps{et}")
                pss.append([t[:, 0:512], t[:, 512:1024]])
            pss.append([ppsum.tile([P, 512], F32, tag="proj", name=f"{pfx}3a")[:],
                        ppsum.tile([P, 512], F32, tag="proj", name=f"{pfx}3b")[:]])
            for dt_ in range(NDT):
                for et in range(NET):
                    for ih in range(NH):
                        nc.tensor.matmul(
                            pss[et][ih],
                            lhsT=w_sb[dt_][:, et * P:(et + 1) * P],
                            rhs=xin[dt_][:, ih * 512:(ih + 1) * 512],
                            start=(dt_ == 0),
                            stop=(dt_ == NDT - 1),
                        )
            # epi_ih_major: S block 0 needs only the ih=0 halves of qT,
            # so drain those four groups first
            if epi_ih_major:
                order = [(et, ih) for ih in range(NH) for et in range(NET)]
            else:
                order = [(et, ih) for et in range(NET) for ih in range(NH)]
            for n, (et, ih) in enumerate(order):
                relu_epilogue(pss[et][ih], bias_sb, out_tiles, et, ih,
                              on_dve=(split_epi and n % 2 == 1))

        def mask_add(kraw, mask_sb, b):
            kTm = [actpool.tile([P, LK], BF16, tag=f"kTm{et}",
                                name=f"kTm{et}_{b}")
                   for et in range(NET)]
            for et in range(NET):
                # split across gpsimd and vector so neither gates the S phase
                eng = nc.gpsimd if et % 2 == 0 else nc.vector
                eng.tensor_add(kTm[et][:], kraw[et][:], mask_sb[:])
            return kTm

        def s_block(b, ib, qTt, kTm, qm_sb):
            sp = spsum.tile([P, LK], F32, tag="S")
            for et in range(NET):
                for jh in range(NH):
                    nc.tensor.matmul(
                        sp[:, jh * 512:(jh + 1) * 512],
                        lhsT=qTt[et][:, ib * P:(ib + 1) * P],
                        rhs=kTm[et][:, jh * 512:(jh + 1) * 512],
                        start=(et == 0),
                        stop=(et == NET - 1),
                    )
            ex = epool.tile([P, LK], BF16, tag="exp")
            rs = stpool.tile([P, 1], F32, tag="rowsum")
            nc.scalar.activation(
                out=ex[:], in_=sp[:], func=AF.Exp, scale=SCALE,
                accum_out=rs[:],
            )
            rc = stpool.tile([P, 1], F32, tag="recip")
            nc.vector.reciprocal(out=rc[:], in_=rs[:])
            rq = stpool.tile([P, 1], F32, tag="rq")
            nc.vector.tensor_tensor(
                out=rq[:], in0=rc[:], in1=qm_sb[:, ib:ib + 1],
                op=mybir.AluOpType.mult,
            )
            po = opool.tile([P, LK], F32, tag="po")
            nc.vector.tensor_scalar(
                out=po[:], in0=ex[:], scalar1=rq[:], scalar2=None,
                op0=mybir.AluOpType.mult,
            )
            # alternate store queues so the output backlog drains 2x faster
            # (sync, not scalar: scalar's ACT must not stall behind DMA issue)
            eng = nc.gpsimd if ib % 2 == 0 else nc.sync
            eng.dma_start(out=out[b, ib * P:(ib + 1) * P, :], in_=po[:])

        def s_block_final(b, ib, qTt, kTm, qm_sb):
            # Last block of the kernel: jh-major matmuls into two separate
            # 1-bank psums + a fully split epilogue (independent half tiles)
            # so the first half's exp/mul/store overlap the second half's
            # matmuls and exp — shortening the serial tail after the last MM.
            sps = [ppsum.tile([P, 512], F32, tag="proj", name=f"fsp{jh}")
                   for jh in range(NH)]
            rss = [stpool.tile([P, 1], F32, tag=f"rowsum{jh}", name=f"frs{jh}")
                   for jh in range(NH)]
            exs = [epool.tile([P, 512], BF16, tag=f"fex{jh}", name=f"fex{jh}")
                   for jh in range(NH)]
            for jh in range(NH):
                for et in range(NET):
                    nc.tensor.matmul(
                        sps[jh][:],
                        lhsT=qTt[et][:, ib * P:(ib + 1) * P],
                        rhs=kTm[et][:, jh * 512:(jh + 1) * 512],
                        start=(et == 0),
                        stop=(et == NET - 1),
                    )
                nc.scalar.activation(
                    out=exs[jh][:], in_=sps[jh][:],
                    func=AF.Exp, scale=SCALE, accum_out=rss[jh][:],
                )
            rs = stpool.tile([P, 1], F32, tag="rowsumt")
            nc.vector.tensor_tensor(
                out=rs[:], in0=rss[0][:], in1=rss[1][:],
                op=mybir.AluOpType.add)
            rc = stpool.tile([P, 1], F32, tag="recip")
            nc.vector.reciprocal(out=rc[:], in_=rs[:])
            rq = stpool.tile([P, 1], F32, tag="rq")
            nc.vector.tensor_tensor(
                out=rq[:], in0=rc[:], in1=qm_sb[:, ib:ib + 1],
                op=mybir.AluOpType.mult,
            )
            for jh in range(NH):
                poh = opool.tile([P, 512], F32, tag=f"fpo{jh}", name=f"fpo{jh}")
                nc.vector.tensor_scalar(
                    out=poh[:], in0=exs[jh][:],
                    scalar1=rq[:], scalar2=None,
                    op0=mybir.AluOpType.mult,
                )
                eng = nc.gpsimd if jh == 0 else nc.sync
                eng.dma_start(
                    out=out[b, ib * P:(ib + 1) * P, jh * 512:(jh + 1) * 512],
                    in_=poh[:],
                )

        def s_phase(b, qTt, kTm, qm_sb):
            for ib in range(NIB):
                if b == BL - 1 and ib == NIB - 1:
                    s_block_final(b, ib, qTt, kTm, qm_sb)
                else:
                    s_block(b, ib, qTt, kTm, qm_sb)

        cur = load_inputs(0)
        for b in range(BL):
            xk, mask_sb, xq, qm_sb = cur
            kraw = [actpool.tile([P, LK], BF16, tag=f"kraw{et}",
                                 name=f"kraw{et}_{b}")
                    for et in range(NET)]
            if b == 0:
                proj_coldstart(xk, wk_sb, bk_sb, kraw, pfx="coldk")
            else:
                proj(xk, wk_sb, bk_sb, kraw)
            kTm = mask_add(kraw, mask_sb, b)
            qTt = [actpool.tile([P, LQ], BF16, tag=f"qT{et}",
                                name=f"qT{et}_{b}")
                   for et in range(NET)]
            if b == 0:
                proj_coldstart(xq, wq_sb, bq_sb, qTt, pfx="coldq", split_epi=True)
            else:
                proj(xq, wq_sb, bq_sb, qTt)
            if b + 1 < BL:
                cur = load_inputs(b + 1)
            s_phase(b, qTt, kTm, qm_sb)


def _build():
    nc = bacc.Bacc(
        "TRN2",
        target_bir_lowering=False,
        debug=False,
        enable_asserts=False,
        num_devices=NCORES,
    )
    qT = nc.dram_tensor("qT", [BL, D, LQ], BF16, kind="ExternalInput").ap()
    kT = nc.dram_tensor("kT", [BL, D, LK], BF16, kind="ExternalInput").ap()
    Wq = nc.dram_tensor("Wq", [D, D], BF16, kind="ExternalInput").ap()
    Wk = nc.dram_tensor("Wk", [D, D], BF16, kind="ExternalInput").ap()
    bq = nc.dram_tensor("bq", [P, NET], F32, kind="ExternalInput").ap()
    bk = nc.dram_tensor("bk", [P, NET], F32, kind="ExternalInput").ap()
    maskb = nc.dram_tensor("maskb", [BL, P, LK], BF16, kind="ExternalInput").ap()
    qm = nc.dram_tensor("qm", [BL, P, NIB], F32, kind="ExternalInput").ap()
    out = nc.dram_tensor("out", [BL, LQ, LK], F32, kind="ExternalOutput").ap()

    with tile.TileContext(nc) as tc:
        _body(tc, qT, kT, Wq, Wk, bq, bk, maskb, qm, out)
    nc.compile()
    return nc


def _get_nc():
    if "nc" not in _CACHE:
        _CACHE["nc"] = _build()
    return _CACHE["nc"]


def _make_in_maps(query, key, query_mask, key_mask, Wq, bq, Wk, bk):
    bf = ml_dtypes.bfloat16
    query = np.asarray(query, dtype=np.float32)
    key = np.asarray(key, dtype=np.float32)
    query_mask = np.asarray(query_mask)
    key_mask = np.asarray(key_mask)
    Wq_b = np.asarray(Wq, dtype=np.float32).astype(bf)
    Wk_b = np.asarray(Wk, dtype=np.float32).astype(bf)
    # bias for feature e lives at partition e%128, column e//128
    bq_t = np.asarray(bq, dtype=np.float32).reshape(NET, P).T.copy()
    bk_t = np.asarray(bk, dtype=np.float32).reshape(NET, P).T.copy()

    in_maps = []
    for c in range(NCORES):
        sl = slice(c * BL, (c + 1) * BL)
        qTc = query[sl].transpose(0, 2, 1).astype(bf)
        kTc = key[sl].transpose(0, 2, 1).astype(bf)
        mrow = (MASKC * (1 - key_mask[sl])).astype(bf)            # [BL, LK]
        maskb = np.ascontiguousarray(
            np.broadcast_to(mrow[:, None, :], (BL, P, LK))
        )
        qmc = (
            query_mask[sl].astype(np.float32)
            .reshape(BL, NIB, P).transpose(0, 2, 1).copy()
        )
        in_maps.append({
            "qT": qTc, "kT": kTc, "Wq": Wq_b, "Wk": Wk_b,
            "bq": bq_t, "bk": bk_t, "maskb": maskb, "qm": qmc,
        })
    return in_maps


def run(query, key, query_mask, key_mask, Wq, bq, Wk, bk, **kwargs):
    """Run on hardware; returns (output, BassKernelResults)."""
    nc = _get_nc()
    in_maps = _make_in_maps(query, key, query_mask, key_mask, Wq, bq, Wk, bk)
    res = run_bass_kernel_spmd(nc, in_maps, core_ids=list(range(NCORES)), **kwargs)
    outs = [res.results[c]["out"] for c in range(NCORES)]
    full = np.concatenate(outs, axis=0).astype(np.float32, copy=False)
    return full, res


def kernel(query, key, query_mask, key_mask, Wq, bq, Wk, bk):
    full, _ = run(query, key, query_mask, key_mask, Wq, bq, Wk, bk)
    return full


# revision 32
# speedup vs baseline: 1.1713x; 1.0485x over previous
"""Masked attention-weight kernel (dense_transformer) for 8 TRN2 NeuronCores.

Computes, for inputs query/key [32,1024,512] f32, masks [32,1024] i32:
    q = relu(query @ Wq + bq); k = relu(key @ Wk + bk)
    w = softmax((q @ k^T)/sqrt(512) + key_mask_additive) * query_mask
Output: [32, 1024, 1024] f32.

Strategy: pure data-parallel over batch (4 batches/core, no collectives).
Host pre-transposes query/key to [B_local, D, L] and casts to bf16 so every
device matmul is transpose-free; compute is bf16 with f32 PSUM accumulation.
Softmax skips max-subtraction (logits bounded ~+-12; exp is safe in f32).
The key mask is applied additively (-1e6) to the post-relu k-projection,
which makes masked logits ~-2e8 so exp underflows to exactly 0, and the
ACT exp's fused accum_out produces the masked row sum for free.

Per-core pipeline, per batch:
  1. kT[e,j] = relu(Wk.T @ keyT + bk) via PE matmuls -> ACT relu+bias,
     then +mask on DVE/GpSimd (batch 0 runs the matmuls dt-major so the PE
     consumes (wk_dt, xk_dt) DMA pairs in arrival order at cold start).
  2. qT[e,i] likewise.
  3. For each 128-row block: S = qT.T @ kTm (PE, f32 psum), ACT exp with
     fused row-sum, DVE reciprocal * query_mask, DVE scale, DMA out
     (stores alternate between the gpsimd and scalar queues).
"""

import sys

sys.path.insert(0, "/opt/trn_rl_repo")

import numpy as np
import ml_dtypes
from contextlib import ExitStack

import concourse.tile as tile
from concourse import bacc, mybir
from concourse.bass_utils import run_bass_kernel_spmd

P = 128
B, LQ, LK, D = 32, 1024, 1024, 512
NCORES = 8
BL = B // NCORES          # batches per core
NDT = D // P              # contraction tiles for projections
NET = D // P              # output-feature tiles (= S contraction tiles)
NIB = LQ // P             # 128-row blocks of S per batch
NH = LK // 512            # 512-col halves
SCALE = float(1.0 / np.sqrt(D))
MASKC = -1.0e6

F32 = mybir.dt.float32
BF16 = mybir.dt.bfloat16
AF = mybir.ActivationFunctionType

_CACHE = {}


def _body(tc, qT, kT, Wq, Wk, bq, bk, maskb, qm, out):
    nc = tc.nc
    with ExitStack() as ctx:
        consts = ctx.enter_context(tc.tile_pool(name="consts", bufs=1))
        wpool = ctx.enter_context(tc.tile_pool(name="w", bufs=1))
        inpool = ctx.enter_context(tc.tile_pool(name="inp", bufs=2))
        actpool = ctx.enter_context(tc.tile_pool(name="act", bufs=2))
        mpool = ctx.enter_context(tc.tile_pool(name="mask", bufs=2))
        epool = ctx.enter_context(tc.tile_pool(name="exp", bufs=3))
        opool = ctx.enter_context(tc.tile_pool(name="pout", bufs=3))
        stpool = ctx.enter_context(tc.tile_pool(name="stat", bufs=6))
        ppsum = ctx.enter_context(tc.tile_pool(name="ppsum", bufs=2, space="PSUM"))
        spsum = ctx.enter_context(tc.tile_pool(name="spsum", bufs=3, space="PSUM"))

        # Weights on the scalar DMA queue, inputs on sync, small tensors on
        # gpsimd — three queues pull concurrently at cold start.
        wk_sb = [wpool.tile([P, D], BF16, tag=f"wk{dt_}", name=f"wk{dt_}")
                 for dt_ in range(NDT)]
        wq_sb = [wpool.tile([P, D], BF16, tag=f"wq{dt_}", name=f"wq{dt_}")
                 for dt_ in range(NDT)]
        for dt_ in range(NDT):
            nc.scalar.dma_start(
                out=wk_sb[dt_][:], in_=Wk[dt_ * P:(dt_ + 1) * P, :])
        for dt_ in range(NDT):
            nc.scalar.dma_start(
                out=wq_sb[dt_][:], in_=Wq[dt_ * P:(dt_ + 1) * P, :])

        bk_sb = consts.tile([P, NET], F32)
        nc.gpsimd.dma_start(out=bk_sb[:], in_=bk[:])
        bq_sb = consts.tile([P, NET], F32)
        nc.gpsimd.dma_start(out=bq_sb[:], in_=bq[:])

        # PE warmup: 8 dummy matmuls (~3.4us of cold PE busy, exactly one
        # HAM activity window) on scratch tiles while the first input DMAs
        # are in flight, so the clock-gate reaches K=8/8 just before real
        # matmuls start. Results are never read.
        warm_in = consts.tile([P, 512], BF16, name="warm_in")
        nc.vector.memset(warm_in[:], 0.0)
        warm_ps = ppsum.tile([P, 512], F32, tag="proj", name="warm_ps")
        for _ in range(8):
            nc.tensor.matmul(
                warm_ps[:], lhsT=warm_in[:, 0:P], rhs=warm_in[:],
                start=True, stop=True,
            )

        def load_inputs(b):
            xk, xq = [], []
            for dt_ in range(NDT):
                t = inpool.tile([P, LK], BF16, tag=f"xk{dt_}")
                if b == 0 and dt_ == 0:
                    # split so the very first matmul's 128KB dep lands sooner
                    for h in range(NH):
                        nc.sync.dma_start(
                            out=t[:, h * 512:(h + 1) * 512],
                            in_=kT[b, 0:P, h * 512:(h + 1) * 512])
                else:
                    nc.sync.dma_start(
                        out=t[:], in_=kT[b, dt_ * P:(dt_ + 1) * P, :])
                xk.append(t)
            mask_sb = mpool.tile([P, LK], BF16, tag="maskb")
            if b > 0:
                # prefetched with plenty of slack; keep off the sync queue
                nc.gpsimd.dma_start(out=mask_sb[:], in_=maskb[b])
            for dt_ in range(NDT):
                t = inpool.tile([P, LQ], BF16, tag=f"xq{dt_}")
                if b == 0 and dt_ == 0:
                    for h in range(NH):
                        nc.sync.dma_start(
                            out=t[:, h * 512:(h + 1) * 512],
                            in_=qT[b, 0:P, h * 512:(h + 1) * 512])
                else:
                    nc.sync.dma_start(
                        out=t[:], in_=qT[b, dt_ * P:(dt_ + 1) * P, :])
                xq.append(t)
            if b == 0:
                # batch 0: issue after xq so the mask transfer doesn't steal
                # bandwidth from the cold-start critical path (wk/xk pairs)
                nc.sync.dma_start(out=mask_sb[:], in_=maskb[b])
            qm_sb = mpool.tile([P, NIB], F32, tag="qm")
            nc.gpsimd.dma_start(out=qm_sb[:], in_=qm[b])
            return xk, mask_sb, xq, qm_sb

        def relu_epilogue(ps, bias_sb, out_tiles, et, ih, on_dve=False):
            if on_dve:
                # (psum + bias) max 0 — exact relu+bias as one DVE op
                nc.vector.tensor_scalar(
                    out=out_tiles[et][:, ih * 512:(ih + 1) * 512],
                    in0=ps[:],
                    scalar1=bias_sb[:, et:et + 1],
                    scalar2=0.0,
                    op0=mybir.AluOpType.add,
                    op1=mybir.AluOpType.max,
                )
            else:
                nc.scalar.activation(
                    out=out_tiles[et][:, ih * 512:(ih + 1) * 512],
                    in_=ps[:],
                    func=AF.Relu,
                    bias=bias_sb[:, et:et + 1],
                    scale=1.0,
                )

        def proj(xin, w_sb, bias_sb, out_tiles):
            # out_tiles[et] = relu(W[:, et].T @ x + b)
            for et in range(NET):
                for ih in range(NH):
                    ps = ppsum.tile([P, 512], F32, tag="proj")
                    for dt_ in range(NDT):
                        nc.tensor.matmul(
                            ps[:],
                            lhsT=w_sb[dt_][:, et * P:(et + 1) * P],
                            rhs=xin[dt_][:, ih * 512:(ih + 1) * 512],
                            start=(dt_ == 0),
                            stop=(dt_ == NDT - 1),
                        )
                    relu_epilogue(ps, bias_sb, out_tiles, et, ih)

        def proj_coldstart(xin, w_sb, bias_sb, out_tiles, pfx="coldk", epi_ih_major=False, split_epi=False):
            # Batch-0 k-proj only: dt-major order so the PE consumes
            # (wk_dt, xk_dt) DMA pairs in arrival order instead of stalling
            # on wk1-3; all 4 et accumulation groups are open at once,
            # borrowing the (still idle) S-phase psum pool for et 0-2.
            pss = []
            for et in range(NET - 1):
                t = spsum.tile([P, LK], F32, tag="S", name=f"{pfx}ps{et}")
                pss.append([t[:, 0:512], t[:, 512:1024]])
            pss.append([ppsum.tile([P, 512], F32, tag="proj", name=f"{pfx}3a")[:],
                        ppsum.tile([P, 512], F32, tag="proj", name=f"{pfx}3b")[:]])
            for dt_ in range(NDT):
                for et in range(NET):
                    for ih in range(NH):
                        nc.tensor.matmul(
                            pss[et][ih],
                            lhsT=w_sb[dt_][:, et * P:(et + 1) * P],
                            rhs=xin[dt_][:, ih * 512:(ih + 1) * 512],
                            start=(dt_ == 0),
                            stop=(dt_ == NDT - 1),
                        )
            # epi_ih_major: S block 0 needs only the ih=0 halves of qT,
            # so drain those four groups first
            if epi_ih_major:
                order = [(et, ih) for ih in range(NH) for et in range(NET)]
            else:
                order = [(et, ih) for et in range(NET) for ih in range(NH)]
            for n, (et, ih) in enumerate(order):
                relu_epilogue(pss[et][ih], bias_sb, out_tiles, et, ih,
                              on_dve=(split_epi and n % 2 == 1))

        def mask_add(kraw, mask_sb, b):
            kTm = [actpool.tile([P, LK], BF16, tag=f"kTm{et}",
                                name=f"kTm{et}_{b}")
                   for et in range(NET)]
            for et in range(NET):
                # split across gpsimd and vector so neither gates the S phase
                eng = nc.gpsimd if et % 2 == 0 else nc.vector
                eng.tensor_add(kTm[et][:], kraw[et][:], mask_sb[:])
            return kTm

        def s_block(b, ib, qTt, kTm, qm_sb):
            sp = spsum.tile([P, LK], F32, tag="S")
            for et in range(NET):
                for jh in range(NH):
                    nc.tensor.matmul(
                        sp[:, jh * 512:(jh + 1) * 512],
                        lhsT=qTt[et][:, ib * P:(ib + 1) * P],
                        rhs=kTm[et][:, jh * 512:(jh + 1) * 512],
                        start=(et == 0),
                        stop=(et == NET - 1),
                    )
            ex = epool.tile([P, LK], BF16, tag="exp")
            rs = stpool.tile([P, 1], F32, tag="rowsum")
            nc.scalar.activation(
                out=ex[:], in_=sp[:], func=AF.Exp, scale=SCALE,
                accum_out=rs[:],
            )
            rc = stpool.tile([P, 1], F32, tag="recip")
            nc.vector.reciprocal(out=rc[:], in_=rs[:])
            rq = stpool.tile([P, 1], F32, tag="rq")
            nc.vector.tensor_tensor(
                out=rq[:], in0=rc[:], in1=qm_sb[:, ib:ib + 1],
                op=mybir.AluOpType.mult,
            )
            po = opool.tile([P, LK], F32, tag="po")
            nc.vector.tensor_scalar(
                out=po[:], in0=ex[:], scalar1=rq[:], scalar2=None,
                op0=mybir.AluOpType.mult,
            )
            # alternate store queues so the output backlog drains 2x faster
            # (sync, not scalar: scalar's ACT must not stall behind DMA issue)
            eng = nc.gpsimd if ib % 2 == 0 else nc.sync
            eng.dma_start(out=out[b, ib * P:(ib + 1) * P, :], in_=po[:])

        def s_block_final(b, ib, qTt, kTm, qm_sb):
            # Last block of the kernel: jh-major matmuls into two separate
            # 1-bank psums + a fully split epilogue (independent half tiles)
            # so the first half's exp/mul/store overlap the second half's
            # matmuls and exp — shortening the serial tail after the last MM.
            sps = [ppsum.tile([P, 512], F32, tag="proj", name=f"fsp{jh}")
                   for jh in range(NH)]
            rss = [stpool.tile([P, 1], F32, tag=f"rowsum{jh}", name=f"frs{jh}")
                   for jh in range(NH)]
            exs = [epool.tile([P, 512], BF16, tag=f"fex{jh}", name=f"fex{jh}")
                   for jh in range(NH)]
            for jh in range(NH):
                for et in range(NET):
                    nc.tensor.matmul(
                        sps[jh][:],
                        lhsT=qTt[et][:, ib * P:(ib + 1) * P],
                        rhs=kTm[et][:, jh * 512:(jh + 1) * 512],
                        start=(et == 0),
                        stop=(et == NET - 1),
                    )
                nc.scalar.activation(
                    out=exs[jh][:], in_=sps[jh][:],
                    func=AF.Exp, scale=SCALE, accum_out=rss[jh][:],
                )
            rs = stpool.tile([P, 1], F32, tag="rowsumt")
            nc.vector.tensor_tensor(
                out=rs[:], in0=rss[0][:], in1=rss[1][:],
                op=mybir.AluOpType.add)
            rc = stpool.tile([P, 1], F32, tag="recip")
            nc.vector.reciprocal(out=rc[:], in_=rs[:])
            rq = stpool.tile([P, 1], F32, tag="rq")
            nc.vector.tensor_tensor(
                out=rq[:], in0=rc[:], in1=qm_sb[:, ib:ib + 1],
                op=mybir.AluOpType.mult,
            )
            for jh in range(NH):
                poh = opool.tile([P, 512], F32, tag=f"fpo{jh}", name=f"fpo{jh}")
                nc.vector.tensor_scalar(
                    out=poh[:], in0=exs[jh][:],
                    scalar1=rq[:], scalar2=None,
                    op0=mybir.AluOpType.mult,
                )
                eng = nc.gpsimd if jh == 0 else nc.sync
                eng.dma_start(
                    out=out[b, ib * P:(ib + 1) * P, jh * 512:(jh + 1) * 512],
                    in_=poh[:],
                )

        def s_phase(b, qTt, kTm, qm_sb):
            for ib in range(NIB):
                if b == BL - 1 and ib == NIB - 1:
                    s_block_final(b, ib, qTt, kTm, qm_sb)
                else:
                    s_block(b, ib, qTt, kTm, qm_sb)

        cur = load_inputs(0)
        for b in range(BL):
            xk, mask_sb, xq, qm_sb = cur
            kraw = [actpool.tile([P, LK], BF16, tag=f"kraw{et}",
                                 name=f"kraw{et}_{b}")
                    for et in range(NET)]
            if b == 0:
                proj_coldstart(xk, wk_sb, bk_sb, kraw, pfx="coldk")
            else:
                proj(xk, wk_sb, bk_sb, kraw)
            kTm = mask_add(kraw, mask_sb, b)
            qTt = [actpool.tile([P, LQ], BF16, tag=f"qT{et}",
                                name=f"qT{et}_{b}")
                   for et in range(NET)]
            if b == 0:
                proj_coldstart(xq, wq_sb, bq_sb, qTt, pfx="coldq", split_epi=True)
            else:
                proj(xq, wq_sb, bq_sb, qTt)
            if b + 1 < BL:
                cur = load_inputs(b + 1)
            s_phase(b, qTt, kTm, qm_sb)


def _build():
    nc = bacc.Bacc(
        "TRN2",
        target_bir_lowering=False,
        debug=False,
        enable_asserts=False,
        num_devices=NCORES,
    )
    qT = nc.dram_tensor("qT", [BL, D, LQ], BF16, kind="ExternalInput").ap()
    kT = nc.dram_tensor("kT", [BL, D, LK], BF16, kind="ExternalInput").ap()
    Wq = nc.dram_tensor("Wq", [D, D], BF16, kind="ExternalInput").ap()
    Wk = nc.dram_tensor("Wk", [D, D], BF16, kind="ExternalInput").ap()
    bq = nc.dram_tensor("bq", [P, NET], F32, kind="ExternalInput").ap()
    bk = nc.dram_tensor("bk", [P, NET], F32, kind="ExternalInput").ap()
    maskb = nc.dram_tensor("maskb", [BL, P, LK], BF16, kind="ExternalInput").ap()
    qm = nc.dram_tensor("qm", [BL, P, NIB], F32, kind="ExternalInput").ap()
    out = nc.dram_tensor("out", [BL, LQ, LK], F32, kind="ExternalOutput").ap()

    with tile.TileContext(nc) as tc:
        _body(tc, qT, kT, Wq, Wk, bq, bk, maskb, qm, out)
    nc.compile()
    return nc


def _get_nc():
    if "nc" not in _CACHE:
        _CACHE["nc"] = _build()
    return _CACHE["nc"]


def _make_in_maps(query, key, query_mask, key_mask, Wq, bq, Wk, bk):
    bf = ml_dtypes.bfloat16
    query = np.asarray(query, dtype=np.float32)
    key = np.asarray(key, dtype=np.float32)
    query_mask = np.asarray(query_mask)
    key_mask = np.asarray(key_mask)
    Wq_b = np.asarray(Wq, dtype=np.float32).astype(bf)
    Wk_b = np.asarray(Wk, dtype=np.float32).astype(bf)
    # bias for feature e lives at partition e%128, column e//128
    bq_t = np.asarray(bq, dtype=np.float32).reshape(NET, P).T.copy()
    bk_t = np.asarray(bk, dtype=np.float32).reshape(NET, P).T.copy()

    in_maps = []
    for c in range(NCORES):
        sl = slice(c * BL, (c + 1) * BL)
        qTc = query[sl].transpose(0, 2, 1).astype(bf)
        kTc = key[sl].transpose(0, 2, 1).astype(bf)
        mrow = (MASKC * (1 - key_mask[sl])).astype(bf)            # [BL, LK]
        maskb = np.ascontiguousarray(
            np.broadcast_to(mrow[:, None, :], (BL, P, LK))
        )
        qmc = (
            query_mask[sl].astype(np.float32)
            .reshape(BL, NIB, P).transpose(0, 2, 1).copy()
        )
        in_maps.append({
            "qT": qTc, "kT": kTc, "Wq": Wq_b, "Wk": Wk_b,
            "bq": bq_t, "bk": bk_t, "maskb": maskb, "qm": qmc,
        })
    return in_maps


def run(query, key, query_mask, key_mask, Wq, bq, Wk, bk, **kwargs):
    """Run on hardware; returns (output, BassKernelResults)."""
    nc = _get_nc()
    in_maps = _make_in_maps(query, key, query_mask, key_mask, Wq, bq, Wk, bk)
    res = run_bass_kernel_spmd(nc, in_maps, core_ids=list(range(NCORES)), **kwargs)
    outs = [res.results[c]["out"] for c in range(NCORES)]
    full = np.concatenate(outs, axis=0).astype(np.float32, copy=False)
    return full, res


def kernel(query, key, query_mask, key_mask, Wq, bq, Wk, bk):
    full, _ = run(query, key, query_mask, key_mask, Wq, bq, Wk, bk)
    return full
